# revision 24
# baseline (speedup 1.0000x reference)
"""Trainium2 Bass kernel: CrossAttention (B=2, Nq=1024, Nkv=2048, D=1024, H=16).

Sharding: 8 cores = 2 (batch) x 4 (head groups of 4 heads).
Each core computes, for its batch b and heads [4g, 4g+4):
    qT = (x_b @ Wq_s)^T            [256, 1024]   (dd on partitions)
    kT = (mem_b @ Wk_s)^T          [256, 2048]
    v  = mem_b @ Wv_s              [2048, 256]   (+ ones column per head)
    per head: sT = k_h @ q_h^T     [2048, 1024]  (kv on partitions)
              eT = exp(SCALE*sT)
              cu = [v_h | 1]^T-acc [65, 1024]    (row 64 = softmax denom)
    per head: P_h = cu[0:64]^T-proj @ Wp_h       [1024, 1024] UNNORMALIZED
Host does the softmax division: out = sum_h P_h / den_h + b_proj.
(Deferring the normalization to the host removes the recip/broadcast/mul
chains that idled the PE at head boundaries and triggered HAM re-throttle.)

v4 scheduling notes:
 - HAM discipline: PE gaps >~1.4us re-throttle the clock to 1.2 GHz and it
   costs ~3.4us of dense work to re-warm.  Dummy matmuls on a memset tile
   warm the PE from t=0 and bridge DMA stalls in the startup phase.
 - DMA order is by first-need: wq/xt stream qT, wk+memt half-0 feed
   kT(0,0), the rest follows.  ones/vones DMAs replaced with memsets.
 - Outputs are bf16 per-head unnormalized projections, DMA'd per-row-chunk
   as soon as each evacuation finishes (no f32 combine on device, no
   serial DVE add tail).
 - ACT only runs Exp (+Copy at the tail): table loaded at t=0.
"""

import numpy as np
import ml_dtypes

DIM = 1024
HEADS = 16
HD = 64
B = 2
NQ = 1024
NKV = 2048
SCALE = HD ** -0.5
N_CORES = 8
HG = 4               # heads per core
DD = HG * HD         # 256 packed head dims per core
KC = 8               # contraction chunks (DIM / 128)
JC = NKV // 128      # 16 kv-row chunks

_CACHE = {}


def _build_module():
    import concourse.bacc as bacc
    import concourse.tile as tile
    import concourse.mybir as mybir

    f32 = mybir.dt.float32
    bf16 = mybir.dt.bfloat16
    EXP = mybir.ActivationFunctionType.Exp

    nc = bacc.Bacc(
        trn_type="TRN2",
        target_bir_lowering=False,
        debug=False,
        num_devices=N_CORES,
    )

    xt_d = nc.dram_tensor("xt", [128, KC, NQ], bf16, kind="ExternalInput").ap()
    memt_d = nc.dram_tensor(
        "memt", [128, 2, KC, NKV // 2], bf16, kind="ExternalInput"
    ).ap()
    wq_d = nc.dram_tensor("wq", [128, KC, DD], bf16, kind="ExternalInput").ap()
    wk_d = nc.dram_tensor("wk", [128, KC, DD], bf16, kind="ExternalInput").ap()
    wv_d = nc.dram_tensor("wv", [128, KC, DD], bf16, kind="ExternalInput").ap()
    wp_d = nc.dram_tensor("wp", [64, HG, DIM], bf16, kind="ExternalInput").ap()
    pout_d = nc.dram_tensor("pout", [HG, NQ, DIM], bf16, kind="ExternalOutput").ap()
    dens_d = nc.dram_tensor("dens", [HG, NQ], bf16, kind="ExternalOutput").ap()

    with tile.TileContext(nc) as tc:
        with (
            tc.tile_pool(name="wpool", bufs=1) as wpool,
            tc.tile_pool(name="persist", bufs=1) as persist,
            tc.tile_pool(name="work", bufs=2) as work,
            tc.tile_pool(name="opool", bufs=8) as opool,
            tc.tile_pool(name="psum", bufs=3, space="PSUM") as psum,
        ):
            # ---- t=0: engine warm-ups, no DMA dependencies ----
            dummy_sb = wpool.tile([128, 512], bf16, name="dummy_sb")
            nc.vector.memset(dummy_sb, 0.125)
            warm_sb = wpool.tile([1, 16], f32, name="warm_sb")
            nc.scalar.activation(out=warm_sb, in_=dummy_sb[0:1, 0:16], func=EXP)
            vaug_sb = persist.tile([128, JC, HG, HD + 1], bf16, name="vaug_sb")
            nc.vector.memset(vaug_sb, 1.0)

            ndum = [0]

            def dummy_mm(n=1):
                # PE heater: keeps the HAM activity window busy while the
                # PE would otherwise idle on DMA (never on the dep chain).
                for _ in range(n):
                    d_ps = psum.tile(
                        [128, 512], f32, name=f"d_ps{ndum[0]}", tag="ps", bufs=3
                    )
                    ndum[0] += 1
                    nc.tensor.matmul(
                        d_ps, lhsT=dummy_sb[:, 0:128], rhs=dummy_sb,
                        start=True, stop=True,
                    )

            dummy_mm(16)

            # ---- DMA issues, ordered by first need (sync = HWDGE FIFO).
            # Critical path to the first exp: qT needs wq+xt (2.5MB), kT(0,0)
            # needs wk+memt half-0 (2.5MB); emission below matches arrival ----
            wq_lo = wpool.tile([128, 4, DD], bf16, name="wq_lo")
            nc.sync.dma_start(out=wq_lo, in_=wq_d[:, 0:4, :])
            xt_t = []
            for i in range(4):
                t = wpool.tile([128, 2, NQ], bf16, name=f"xt{i}")
                xt_t.append(t)
            nc.sync.dma_start(out=xt_t[0], in_=xt_d[:, 0:2, :])
            wk_sb = wpool.tile([128, KC, DD], bf16, name="wk_sb")
            nc.sync.dma_start(out=wk_sb, in_=wk_d)
            memt0a = wpool.tile([128, 4, NKV // 2], bf16, name="memt0a")
            nc.sync.dma_start(out=memt0a, in_=memt_d[:, 0, 0:4, :])
            memt0b = wpool.tile([128, 4, NKV // 2], bf16, name="memt0b")
            nc.sync.dma_start(out=memt0b, in_=memt_d[:, 0, 4:8, :])
            wv_sb = wpool.tile([128, KC, DD], bf16, name="wv_sb")
            nc.sync.dma_start(out=wv_sb, in_=wv_d)
            nc.sync.dma_start(out=xt_t[1], in_=xt_d[:, 2:4, :])
            wq_hi = wpool.tile([128, 4, DD], bf16, name="wq_hi")
            nc.sync.dma_start(out=wq_hi, in_=wq_d[:, 4:8, :])
            nc.sync.dma_start(out=xt_t[2], in_=xt_d[:, 4:6, :])
            nc.sync.dma_start(out=xt_t[3], in_=xt_d[:, 6:8, :])
            memt1a = wpool.tile([128, KC, 512], bf16, name="memt1a")
            nc.sync.dma_start(out=memt1a, in_=memt_d[:, 1, :, 0:512])
            memt1b = wpool.tile([128, KC, 512], bf16, name="memt1b")
            nc.sync.dma_start(out=memt1b, in_=memt_d[:, 1, :, 512:1024])
            wp_sb = wpool.tile([64, HG, DIM], bf16, name="wp_sb")
            nc.sync.dma_start(out=wp_sb, in_=wp_d)

            def memt(jh2, kc, lo, hi):
                # column window [lo, hi) within the 1024-wide kv half
                if jh2 == 0:
                    t = memt0a if kc < 4 else memt0b
                    return t[:, kc % 4, lo:hi]
                if hi <= 512:
                    return memt1a[:, kc, lo:hi]
                return memt1b[:, kc, lo - 512 : hi - 512]

            def wq(kc):
                return wq_lo[:, kc, :] if kc < 4 else wq_hi[:, kc - 4, :]

            # ---- persistent intermediates ----
            qT_sb = persist.tile([128, 2, NQ], bf16, name="qT_sb")
            kT_sb = persist.tile([128, 2, NKV], bf16, name="kT_sb")
            ctxr = [None] * HG
            cu = [None] * HG

            kt_state = {}

            def kt_tile(mc, jh2, tag="ps"):
                key = (mc, jh2)
                kt_ps = kt_state.get(key)
                if kt_ps is None:
                    kt_ps = psum.tile(
                        [128, NKV // 2], f32, name=f"kt_ps_{mc}_{jh2}",
                        tag=tag, bufs=(1 if tag == "cu" else 3),
                    )
                    kt_state[key] = kt_ps
                return kt_ps

            def kt_sub(mc, jh2, ki, tag="ps"):
                # kT quarter in kc-pair granules (4 mms each)
                kt_ps = kt_tile(mc, jh2, tag)
                for kc in (2 * ki, 2 * ki + 1):
                    for jh in range(2):
                        nc.tensor.matmul(
                            kt_ps[:, jh * 512 : (jh + 1) * 512],
                            lhsT=wk_sb[:, kc, mc * 128 : (mc + 1) * 128],
                            rhs=memt(jh2, kc, jh * 512, (jh + 1) * 512),
                            start=(kc == 0),
                            stop=(kc == KC - 1),
                        )

            def kt_sub_jh(mc, jh2, jh, tag="ps"):
                # kT quarter in jh-major granules (8 mms, one column bank):
                # lets kv-half-1 work start as soon as memt1a lands
                kt_ps = kt_tile(mc, jh2, tag)
                for kc in range(KC):
                    nc.tensor.matmul(
                        kt_ps[:, jh * 512 : (jh + 1) * 512],
                        lhsT=wk_sb[:, kc, mc * 128 : (mc + 1) * 128],
                        rhs=memt(jh2, kc, jh * 512, (jh + 1) * 512),
                        start=(kc == 0),
                        stop=(kc == KC - 1),
                    )

            def kt_copy(mc, jh2):
                kt_ps = kt_state.pop((mc, jh2))
                nc.vector.tensor_copy(
                    out=kT_sb[:, mc, jh2 * 1024 : (jh2 + 1) * 1024], in_=kt_ps
                )

            # ---- qT + kT(0,0), interleaved to match DMA arrival order:
            # qt kc0-1 (wq_lo+xt0), kT(0,0) (wk+memt half-0, parked in the
            # idle cu psum slot), then qt kc2-7 as the xt chunks land ----
            qt_ps = [
                psum.tile([128, NQ], f32, name=f"qt_ps{mc}", tag="ps", bufs=3)
                for mc in range(2)
            ]

            def qt_kc(kc, mcs=(0, 1)):
                xs = xt_t[kc // 2][:, kc % 2, :]
                for mc in mcs:
                    for ih in range(2):
                        nc.tensor.matmul(
                            qt_ps[mc][:, ih * 512 : (ih + 1) * 512],
                            lhsT=wq(kc)[:, mc * 128 : (mc + 1) * 128],
                            rhs=xs[:, ih * 512 : (ih + 1) * 512],
                            start=(kc == 0),
                            stop=(kc == KC - 1),
                        )

            def v_chunk(jc):
                jh2, jr = divmod(jc, 8)
                v_ps = psum.tile([128, DD], f32, name=f"v_ps{jc}", tag="ps", bufs=3)
                for kc in range(KC):
                    nc.tensor.matmul(
                        v_ps,
                        lhsT=memt(jh2, kc, jr * 128, (jr + 1) * 128),
                        rhs=wv_sb[:, kc, :],
                        start=(kc == 0),
                        stop=(kc == KC - 1),
                    )
                nc.vector.tensor_copy(
                    out=vaug_sb[:, jc, :, 0:HD],
                    in_=v_ps.rearrange("p (h d) -> p h d", h=HG),
                )

            def qk_exp(h, jc):
                hp = h // 2
                po = (h % 2) * 64
                sT = psum.tile(
                    [128, NQ], f32, name=f"sT_ps_{h}_{jc}", tag="ps", bufs=3
                )
                for ih in range(2):
                    nc.tensor.matmul(
                        sT[:, ih * 512 : (ih + 1) * 512],
                        lhsT=kT_sb[po : po + 64, hp, jc * 128 : (jc + 1) * 128],
                        rhs=qT_sb[po : po + 64, hp, ih * 512 : (ih + 1) * 512],
                        start=True,
                        stop=True,
                    )
                tag = "eT3" if h == 3 else "eT"
                eT = work.tile(
                    [128, NQ], bf16, name="eT_sb", tag=tag,
                    bufs=(JC if h == 3 else 6),
                )
                nc.scalar.activation(out=eT, in_=sT, func=EXP, scale=SCALE)
                return eT

            def pv(h, jc, eT):
                for ih in range(2):
                    nc.tensor.matmul(
                        cu[h][:, ih * 512 : (ih + 1) * 512],
                        lhsT=vaug_sb[:, jc, h, :],
                        rhs=eT[:, ih * 512 : (ih + 1) * 512],
                        start=(jc == 0),
                        stop=(jc == JC - 1),
                    )

            def ctx_evac(h):
                # single CAST frees the cu psum slot; row 64 is the denom
                ctxr[h] = persist.tile([65, NQ], bf16, name=f"ctxr{h}")
                nc.vector.tensor_copy(out=ctxr[h], in_=cu[h])

            def den_dma(h):
                nc.sync.dma_start(out=dens_d[h, :], in_=ctxr[h][64:65, :])

            def proj(h, ic, evac="vector"):
                p_ps = psum.tile(
                    [128, DIM], f32, name=f"p_ps_{h}_{ic}", tag="ps", bufs=3
                )
                for nh in range(2):
                    nc.tensor.matmul(
                        p_ps[:, nh * 512 : (nh + 1) * 512],
                        lhsT=ctxr[h][0:64, ic * 128 : (ic + 1) * 128],
                        rhs=wp_sb[:, h, nh * 512 : (nh + 1) * 512],
                        start=True,
                        stop=True,
                    )
                o_sb = opool.tile([128, DIM], bf16, name="o_sb", tag="out")
                if evac == "vector":
                    nc.vector.tensor_copy(out=o_sb, in_=p_ps)
                else:
                    nc.scalar.copy(out=o_sb, in_=p_ps)
                nc.sync.dma_start(
                    out=pout_d[h, ic * 128 : (ic + 1) * 128, :], in_=o_sb
                )

            # ---- startup emission: qt kc0-1, kT(0,0), then v chunks 0-5
            # fill the rest of the DMA-bound window (wv lands right after
            # memt half-0); qt resumes per xt arrival ----
            qt_kc(0)
            qt_kc(1)
            dummy_mm(1)
            for ki in range(4):
                kt_sub(0, 0, ki, tag="cu")
                if ki == 1:
                    dummy_mm(1)
            kt_copy(0, 0)
            dummy_mm(1)
            v_chunk(0)
            v_chunk(1)
            qt_kc(2)
            qt_kc(3)
            v_chunk(2)
            v_chunk(3)
            qt_kc(4)
            qt_kc(5)
            qt_kc(6)
            # mc0 half of kc7 first: h0 only needs the mc=0 cast
            qt_kc(KC - 1, mcs=(0,))
            nc.vector.tensor_copy(out=qT_sb[:, 0, :], in_=qt_ps[0])
            qt_kc(KC - 1, mcs=(1,))
            nc.vector.tensor_copy(out=qT_sb[:, 1, :], in_=qt_ps[1])
            v_chunk(4)
            v_chunk(5)

            # ---- attention: heads sequential, PE filler interleaved ----
            def attn_head(h, filler):
                cu[h] = psum.tile(
                    [HD + 1, NQ], f32, name=f"cu_ps{h}", tag="cu", bufs=1
                )
                pend = []
                for jc in range(JC):
                    eT = qk_exp(h, jc)
                    for fn in filler.get(jc, ()):
                        fn()
                    pend.append((jc, eT))
                    if jc >= 2:
                        j2, e2 = pend.pop(0)
                        pv(h, j2, e2)
                for j2, e2 in pend:
                    pv(h, j2, e2)

            # h0: remaining v chunks; kT(0,1) in jh-major granules paced by
            # the split memt half-1 arrival (the copy must precede QK(0,8))
            attn_head(0, {
                0: [lambda: v_chunk(6)],
                1: [lambda: v_chunk(7)],
                2: [lambda: v_chunk(8)],
                3: [lambda: v_chunk(9)],
                4: [lambda: v_chunk(10), lambda: kt_sub_jh(0, 1, 0)],
                5: [lambda: v_chunk(11)],
                6: [lambda: kt_sub_jh(0, 1, 1)],
                7: [lambda: kt_copy(0, 1)],
                8: [lambda: v_chunk(12)],
                9: [lambda: v_chunk(13)],
                10: [lambda: v_chunk(14)],
                11: [lambda: v_chunk(15)],
            })

            # h1 (odd head: QK uses PE rows 64-127, so the row 0-63 proj
            # matmuls overlap on the array): evac h0, kT(1,0), proj h0
            attn_head(1, {
                0: [lambda: ctx_evac(0), lambda: kt_sub(1, 0, 0)],
                1: [lambda: den_dma(0), lambda: kt_sub(1, 0, 1)],
                2: [lambda: kt_sub(1, 0, 2)],
                3: [lambda: kt_sub(1, 0, 3), lambda: kt_copy(1, 0)],
                4: [lambda: proj(0, 0)],
                5: [lambda: proj(0, 1)],
                6: [lambda: proj(0, 2)],
                7: [lambda: proj(0, 3)],
                8: [lambda: proj(0, 4)],
                9: [lambda: proj(0, 5)],
                10: [lambda: proj(0, 6)],
                11: [lambda: proj(0, 7)],
            })

            # h2 (even): evac h1, kT(1,1); otherwise ACT-paced
            attn_head(2, {
                0: [lambda: ctx_evac(1), lambda: kt_sub(1, 1, 0)],
                1: [lambda: den_dma(1), lambda: kt_sub(1, 1, 1)],
                2: [lambda: kt_sub(1, 1, 2)],
                3: [lambda: kt_sub(1, 1, 3), lambda: kt_copy(1, 1)],
            })

            # h3 (odd): evac h2, h1 + h2 projections (again row-overlapped).
            # PV is split by column bank: half A (q 0-511) accumulates in the
            # loop, half B after it — so half A's evac + projections overlap
            # the half-B PV matmuls instead of serializing after the last exp.
            filler3 = {
                0: [lambda: ctx_evac(2), lambda: proj(1, 0)],
                1: [lambda: den_dma(2), lambda: proj(1, 1)],
                2: [lambda: proj(1, 2, evac="scalar")],
                3: [lambda: proj(1, 3)],
                4: [lambda: proj(1, 4, evac="scalar")],
                5: [lambda: proj(1, 5)],
                6: [lambda: proj(1, 6, evac="scalar")],
                7: [lambda: proj(1, 7)],
                8: [lambda: proj(2, 0, evac="scalar")],
                9: [lambda: proj(2, 1)],
                10: [lambda: proj(2, 2)],
                11: [lambda: proj(2, 3)],
                12: [lambda: proj(2, 4)],
                13: [lambda: proj(2, 5)],
                14: [lambda: proj(2, 6)],
                15: [lambda: proj(2, 7)],
            }
            cu[3] = psum.tile([HD + 1, NQ], f32, name="cu_ps3", tag="cu", bufs=1)

            def pv3_half(jc, eT, ih):
                nc.tensor.matmul(
                    cu[3][:, ih * 512 : (ih + 1) * 512],
                    lhsT=vaug_sb[:, jc, 3, :],
                    rhs=eT[:, ih * 512 : (ih + 1) * 512],
                    start=(jc == 0),
                    stop=(jc == JC - 1),
                )

            eT3 = []
            pend = []
            for jc in range(JC):
                eT3.append(qk_exp(3, jc))
                for fn in filler3.get(jc, ()):
                    fn()
                pend.append(jc)
                if jc >= 2:
                    j2 = pend.pop(0)
                    pv3_half(j2, eT3[j2], 0)
            for j2 in pend:
                pv3_half(j2, eT3[j2], 0)

            # half A complete: evac + den while PE runs PV half B
            ctxr[3] = persist.tile([65, NQ], bf16, name="ctxr3")
            nc.vector.tensor_copy(out=ctxr[3][:, 0:512], in_=cu[3][:, 0:512])
            nc.sync.dma_start(out=dens_d[3, 0:512], in_=ctxr[3][64:65, 0:512])
            for jc in range(JC):
                pv3_half(jc, eT3[jc], 1)
                if jc == 5:
                    proj(3, 0)
                elif jc == 9:
                    proj(3, 1, evac="scalar")
                elif jc == 13:
                    proj(3, 2)
            nc.vector.tensor_copy(out=ctxr[3][:, 512:1024], in_=cu[3][:, 512:1024])
            nc.sync.dma_start(out=dens_d[3, 512:1024], in_=ctxr[3][64:65, 512:1024])
            proj(3, 3, evac="scalar")
            dummy_mm(2)
            for ic in range(4, 8):
                proj(3, ic, evac=("vector" if ic % 2 == 0 else "scalar"))

    nc.compile()
    return nc


def get_module():
    if "nc" not in _CACHE:
        _CACHE["nc"] = _build_module()
    return _CACHE["nc"]


def make_in_maps(x, mem, W_kv, W_q, W_proj):
    """Host-side shard + repack into the k-major bf16 layouts."""
    bf = ml_dtypes.bfloat16
    x = np.ascontiguousarray(np.asarray(x, np.float32))
    mem = np.ascontiguousarray(np.asarray(mem, np.float32))
    W_kv = np.asarray(W_kv, np.float32)
    W_q = np.asarray(W_q, np.float32)
    W_proj = np.asarray(W_proj, np.float32)

    def pack_k(a):  # [1024, N] -> [128, 8, N] bf16, k-chunked
        n = a.shape[1]
        return (
            np.ascontiguousarray(a.reshape(KC, 128, n).transpose(1, 0, 2))
            .astype(bf)
        )

    xt_b = [pack_k(x[b].T) for b in range(B)]
    # memt: [128, jh2, kc, NKV//2], kv-half major
    memt_b = [
        np.ascontiguousarray(
            pack_k(mem[b].T).reshape(128, KC, 2, NKV // 2).transpose(0, 2, 1, 3)
        )
        for b in range(B)
    ]

    in_maps = []
    for core in range(N_CORES):
        b, g = divmod(core, 4)
        cs = slice(g * DD, (g + 1) * DD)
        wq = pack_k(W_q[:, cs])
        wk = pack_k(W_kv[:, :DIM][:, cs])
        wv = pack_k(W_kv[:, DIM:][:, cs])
        # wp[d, h, n] = W_proj[g*256 + h*64 + d, n]
        wp = (
            np.ascontiguousarray(
                W_proj[cs, :].reshape(HG, HD, DIM).transpose(1, 0, 2)
            )
            .astype(bf)
        )
        in_maps.append(
            {
                "xt": xt_b[b],
                "memt": memt_b[b],
                "wq": wq,
                "wk": wk,
                "wv": wv,
                "wp": wp,
            }
        )
    return in_maps


def combine_outputs(results, b_proj):
    """Host: per-head softmax division + sum partials + bias."""
    b_proj = np.asarray(b_proj, np.float32)
    out = np.zeros((B, NQ, DIM), np.float32)
    for core in range(N_CORES):
        pout = np.asarray(results[core]["pout"], np.float32)  # [HG, NQ, DIM]
        dens = np.asarray(results[core]["dens"], np.float32)  # [HG, NQ]
        out[core // 4] += np.einsum("hqd,hq->qd", pout, 1.0 / dens)
    out += b_proj[None, None, :]
    return out


def kernel(x, mem, W_kv, W_q, W_proj, b_proj):
    from concourse import bass_utils

    nc = get_module()
    in_maps = make_in_maps(x, mem, W_kv, W_q, W_proj)
    res = bass_utils.run_bass_kernel_spmd(
        nc, in_maps, core_ids=list(range(N_CORES))
    )
    return combine_outputs([res.results[c] for c in range(N_CORES)], b_proj)


# revision 29
# speedup vs baseline: 1.0416x; 1.0416x over previous
"""Trainium2 Bass kernel: CrossAttention (B=2, Nq=1024, Nkv=2048, D=1024, H=16).

Sharding: 8 cores = 2 (batch) x 4 (head groups of 4 heads).
Each core computes, for its batch b and heads [4g, 4g+4):
    qT = (x_b @ Wq_s)^T            [256, 1024]   (dd on partitions)
    kT = (mem_b @ Wk_s)^T          [256, 2048]
    v  = mem_b @ Wv_s              [2048, 256]   (+ ones column per head)
    per head: sT = k_h @ q_h^T     [2048, 1024]  (kv on partitions)
              eT = exp(SCALE*sT)
              cu = [v_h | 1]^T-acc [65, 1024]    (row 64 = softmax denom)
    per head: P_h = cu[0:64]^T-proj @ Wp_h       [1024, 1024] UNNORMALIZED
Host does the softmax division: out = sum_h P_h / den_h + b_proj.
(Deferring the normalization to the host removes the recip/broadcast/mul
chains that idled the PE at head boundaries and triggered HAM re-throttle.)

v4 scheduling notes:
 - HAM discipline: PE gaps >~1.4us re-throttle the clock to 1.2 GHz and it
   costs ~3.4us of dense work to re-warm.  Dummy matmuls on a memset tile
   warm the PE from t=0 and bridge DMA stalls in the startup phase.
 - DMA order is by first-need: wq/xt stream qT, wk+memt half-0 feed
   kT(0,0), the rest follows.  ones/vones DMAs replaced with memsets.
 - Outputs are bf16 per-head unnormalized projections, DMA'd per-row-chunk
   as soon as each evacuation finishes (no f32 combine on device, no
   serial DVE add tail).
 - ACT only runs Exp (+Copy at the tail): table loaded at t=0.
"""

import numpy as np
import ml_dtypes

DIM = 1024
HEADS = 16
HD = 64
B = 2
NQ = 1024
NKV = 2048
SCALE = HD ** -0.5
N_CORES = 8
HG = 4               # heads per core
DD = HG * HD         # 256 packed head dims per core
KC = 8               # contraction chunks (DIM / 128)
JC = NKV // 128      # 16 kv-row chunks

_CACHE = {}


def _build_module():
    import concourse.bacc as bacc
    import concourse.tile as tile
    import concourse.mybir as mybir

    f32 = mybir.dt.float32
    bf16 = mybir.dt.bfloat16
    EXP = mybir.ActivationFunctionType.Exp

    nc = bacc.Bacc(
        trn_type="TRN2",
        target_bir_lowering=False,
        debug=False,
        num_devices=N_CORES,
    )

    xt_d = nc.dram_tensor("xt", [128, KC, NQ], bf16, kind="ExternalInput").ap()
    memt_d = nc.dram_tensor(
        "memt", [128, 2, KC, NKV // 2], bf16, kind="ExternalInput"
    ).ap()
    wq_d = nc.dram_tensor("wq", [128, KC, DD], bf16, kind="ExternalInput").ap()
    wk_d = nc.dram_tensor("wk", [128, KC, DD], bf16, kind="ExternalInput").ap()
    wv_d = nc.dram_tensor("wv", [128, KC, DD], bf16, kind="ExternalInput").ap()
    wp_d = nc.dram_tensor("wp", [64, HG, DIM], bf16, kind="ExternalInput").ap()
    pout_d = nc.dram_tensor("pout", [HG, NQ, DIM], bf16, kind="ExternalOutput").ap()
    dens_d = nc.dram_tensor("dens", [HG, NQ], bf16, kind="ExternalOutput").ap()

    with tile.TileContext(nc) as tc:
        with (
            tc.tile_pool(name="wpool", bufs=1) as wpool,
            tc.tile_pool(name="persist", bufs=1) as persist,
            tc.tile_pool(name="work", bufs=2) as work,
            tc.tile_pool(name="opool", bufs=8) as opool,
            tc.tile_pool(name="psum", bufs=3, space="PSUM") as psum,
        ):
            # ---- t=0: engine warm-ups, no DMA dependencies ----
            dummy_sb = wpool.tile([128, 512], bf16, name="dummy_sb")
            nc.vector.memset(dummy_sb, 0.125)
            warm_sb = wpool.tile([1, 16], f32, name="warm_sb")
            nc.scalar.activation(out=warm_sb, in_=dummy_sb[0:1, 0:16], func=EXP)
            vaug_sb = persist.tile([128, JC, HG, HD + 1], bf16, name="vaug_sb")
            nc.vector.memset(vaug_sb, 1.0)

            ndum = [0]

            def dummy_mm(n=1):
                # PE heater: keeps the HAM activity window busy while the
                # PE would otherwise idle on DMA (never on the dep chain).
                for _ in range(n):
                    d_ps = psum.tile(
                        [128, 512], f32, name=f"d_ps{ndum[0]}", tag="ps", bufs=3
                    )
                    ndum[0] += 1
                    nc.tensor.matmul(
                        d_ps, lhsT=dummy_sb[:, 0:128], rhs=dummy_sb,
                        start=True, stop=True,
                    )

            dummy_mm(19)

            # ---- DMA issues, ordered by first need (sync = HWDGE FIFO).
            # Critical path to the first exp: qT needs wq+xt (2.5MB), kT(0,0)
            # needs wk+memt half-0 (2.5MB); emission below matches arrival ----
            wq_lo = wpool.tile([128, 4, DD], bf16, name="wq_lo")
            nc.sync.dma_start(out=wq_lo, in_=wq_d[:, 0:4, :])
            xt_t = []
            for i in range(4):
                t = wpool.tile([128, 2, NQ], bf16, name=f"xt{i}")
                xt_t.append(t)
            nc.sync.dma_start(out=xt_t[0], in_=xt_d[:, 0:2, :])
            wk_sb = wpool.tile([128, KC, DD], bf16, name="wk_sb")
            nc.sync.dma_start(out=wk_sb, in_=wk_d)
            memt0a = wpool.tile([128, 4, NKV // 2], bf16, name="memt0a")
            nc.sync.dma_start(out=memt0a, in_=memt_d[:, 0, 0:4, :])
            memt0b = wpool.tile([128, 4, NKV // 2], bf16, name="memt0b")
            nc.sync.dma_start(out=memt0b, in_=memt_d[:, 0, 4:8, :])
            wv_sb = wpool.tile([128, KC, DD], bf16, name="wv_sb")
            nc.sync.dma_start(out=wv_sb, in_=wv_d)
            nc.sync.dma_start(out=xt_t[1], in_=xt_d[:, 2:4, :])
            wq_hi = wpool.tile([128, 4, DD], bf16, name="wq_hi")
            nc.sync.dma_start(out=wq_hi, in_=wq_d[:, 4:8, :])
            nc.sync.dma_start(out=xt_t[2], in_=xt_d[:, 4:6, :])
            nc.sync.dma_start(out=xt_t[3], in_=xt_d[:, 6:8, :])
            memt1a = wpool.tile([128, KC, 512], bf16, name="memt1a")
            nc.sync.dma_start(out=memt1a, in_=memt_d[:, 1, :, 0:512])
            memt1b = wpool.tile([128, KC, 512], bf16, name="memt1b")
            nc.sync.dma_start(out=memt1b, in_=memt_d[:, 1, :, 512:1024])
            wp_sb = wpool.tile([64, HG, DIM], bf16, name="wp_sb")
            nc.sync.dma_start(out=wp_sb, in_=wp_d)

            def memt(jh2, kc, lo, hi):
                # column window [lo, hi) within the 1024-wide kv half
                if jh2 == 0:
                    t = memt0a if kc < 4 else memt0b
                    return t[:, kc % 4, lo:hi]
                if hi <= 512:
                    return memt1a[:, kc, lo:hi]
                return memt1b[:, kc, lo - 512 : hi - 512]

            def wq(kc):
                return wq_lo[:, kc, :] if kc < 4 else wq_hi[:, kc - 4, :]

            # ---- persistent intermediates ----
            qT_sb = persist.tile([128, 2, NQ], bf16, name="qT_sb")
            kT_sb = persist.tile([128, 2, NKV], bf16, name="kT_sb")
            ctxr = [None] * HG
            cu = [None] * HG

            kt_state = {}

            def kt_tile(mc, jh2, tag="ps"):
                key = (mc, jh2)
                kt_ps = kt_state.get(key)
                if kt_ps is None:
                    kt_ps = psum.tile(
                        [128, NKV // 2], f32, name=f"kt_ps_{mc}_{jh2}",
                        tag=tag, bufs=(1 if tag == "cu" else 3),
                    )
                    kt_state[key] = kt_ps
                return kt_ps

            def kt_sub(mc, jh2, ki, tag="ps"):
                # kT quarter in kc-pair granules (4 mms each)
                kt_ps = kt_tile(mc, jh2, tag)
                for kc in (2 * ki, 2 * ki + 1):
                    for jh in range(2):
                        nc.tensor.matmul(
                            kt_ps[:, jh * 512 : (jh + 1) * 512],
                            lhsT=wk_sb[:, kc, mc * 128 : (mc + 1) * 128],
                            rhs=memt(jh2, kc, jh * 512, (jh + 1) * 512),
                            start=(kc == 0),
                            stop=(kc == KC - 1),
                        )

            def kt_sub_jh(mc, jh2, jh, tag="ps"):
                # kT quarter in jh-major granules (8 mms, one column bank):
                # lets kv-half-1 work start as soon as memt1a lands
                kt_ps = kt_tile(mc, jh2, tag)
                for kc in range(KC):
                    nc.tensor.matmul(
                        kt_ps[:, jh * 512 : (jh + 1) * 512],
                        lhsT=wk_sb[:, kc, mc * 128 : (mc + 1) * 128],
                        rhs=memt(jh2, kc, jh * 512, (jh + 1) * 512),
                        start=(kc == 0),
                        stop=(kc == KC - 1),
                    )

            def kt_copy(mc, jh2):
                kt_ps = kt_state.pop((mc, jh2))
                nc.vector.tensor_copy(
                    out=kT_sb[:, mc, jh2 * 1024 : (jh2 + 1) * 1024], in_=kt_ps
                )

            # ---- qT + kT(0,0), interleaved to match DMA arrival order:
            # qt kc0-1 (wq_lo+xt0), kT(0,0) (wk+memt half-0, parked in the
            # idle cu psum slot), then qt kc2-7 as the xt chunks land ----
            qt_ps = [
                psum.tile([128, NQ], f32, name=f"qt_ps{mc}", tag="ps", bufs=3)
                for mc in range(2)
            ]

            def qt_kc(kc, mcs=(0, 1)):
                xs = xt_t[kc // 2][:, kc % 2, :]
                for mc in mcs:
                    for ih in range(2):
                        nc.tensor.matmul(
                            qt_ps[mc][:, ih * 512 : (ih + 1) * 512],
                            lhsT=wq(kc)[:, mc * 128 : (mc + 1) * 128],
                            rhs=xs[:, ih * 512 : (ih + 1) * 512],
                            start=(kc == 0),
                            stop=(kc == KC - 1),
                        )

            def v_chunk(jc):
                jh2, jr = divmod(jc, 8)
                v_ps = psum.tile([128, DD], f32, name=f"v_ps{jc}", tag="ps", bufs=3)
                for kc in range(KC):
                    nc.tensor.matmul(
                        v_ps,
                        lhsT=memt(jh2, kc, jr * 128, (jr + 1) * 128),
                        rhs=wv_sb[:, kc, :],
                        start=(kc == 0),
                        stop=(kc == KC - 1),
                    )
                nc.vector.tensor_copy(
                    out=vaug_sb[:, jc, :, 0:HD],
                    in_=v_ps.rearrange("p (h d) -> p h d", h=HG),
                )

            def qk_exp(h, jc):
                hp = h // 2
                po = (h % 2) * 64
                sT = psum.tile(
                    [128, NQ], f32, name=f"sT_ps_{h}_{jc}", tag="ps", bufs=3
                )
                for ih in range(2):
                    nc.tensor.matmul(
                        sT[:, ih * 512 : (ih + 1) * 512],
                        lhsT=kT_sb[po : po + 64, hp, jc * 128 : (jc + 1) * 128],
                        rhs=qT_sb[po : po + 64, hp, ih * 512 : (ih + 1) * 512],
                        start=True,
                        stop=True,
                    )
                eT = work.tile([128, NQ], bf16, name="eT_sb", tag="eT", bufs=6)
                nc.scalar.activation(out=eT, in_=sT, func=EXP, scale=SCALE)
                return eT

            def pv(h, jc, eT):
                for ih in range(2):
                    nc.tensor.matmul(
                        cu[h][:, ih * 512 : (ih + 1) * 512],
                        lhsT=vaug_sb[:, jc, h, :],
                        rhs=eT[:, ih * 512 : (ih + 1) * 512],
                        start=(jc == 0),
                        stop=(jc == JC - 1),
                    )

            def ctx_evac(h):
                # single CAST frees the cu psum slot; row 64 is the denom
                ctxr[h] = persist.tile([65, NQ], bf16, name=f"ctxr{h}")
                nc.vector.tensor_copy(out=ctxr[h], in_=cu[h])

            def den_dma(h):
                nc.sync.dma_start(out=dens_d[h, :], in_=ctxr[h][64:65, :])

            def proj(h, ic, evac="vector"):
                p_ps = psum.tile(
                    [128, DIM], f32, name=f"p_ps_{h}_{ic}", tag="ps", bufs=3
                )
                for nh in range(2):
                    nc.tensor.matmul(
                        p_ps[:, nh * 512 : (nh + 1) * 512],
                        lhsT=ctxr[h][0:64, ic * 128 : (ic + 1) * 128],
                        rhs=wp_sb[:, h, nh * 512 : (nh + 1) * 512],
                        start=True,
                        stop=True,
                    )
                o_sb = opool.tile([128, DIM], bf16, name="o_sb", tag="out")
                if evac == "vector":
                    nc.vector.tensor_copy(out=o_sb, in_=p_ps)
                else:
                    nc.scalar.copy(out=o_sb, in_=p_ps)
                nc.sync.dma_start(
                    out=pout_d[h, ic * 128 : (ic + 1) * 128, :], in_=o_sb
                )

            # ---- startup emission: qt kc0-1, kT(0,0), then v chunks 0-5
            # fill the rest of the DMA-bound window (wv lands right after
            # memt half-0); qt resumes per xt arrival ----
            qt_kc(0)
            qt_kc(1)
            dummy_mm(1)
            for ki in range(4):
                kt_sub(0, 0, ki, tag="cu")
                if ki == 1:
                    dummy_mm(1)
            kt_copy(0, 0)
            dummy_mm(1)
            v_chunk(0)
            v_chunk(1)
            qt_kc(2)
            qt_kc(3)
            v_chunk(2)
            v_chunk(3)
            qt_kc(4)
            qt_kc(5)
            qt_kc(6)
            # mc0 half of kc7 first: h0 only needs the mc=0 cast
            qt_kc(KC - 1, mcs=(0,))
            nc.vector.tensor_copy(out=qT_sb[:, 0, :], in_=qt_ps[0])
            qt_kc(KC - 1, mcs=(1,))
            nc.vector.tensor_copy(out=qT_sb[:, 1, :], in_=qt_ps[1])
            v_chunk(4)
            v_chunk(5)

            # ---- attention: heads sequential, PE filler interleaved ----
            def attn_head(h, filler):
                cu[h] = psum.tile(
                    [HD + 1, NQ], f32, name=f"cu_ps{h}", tag="cu", bufs=1
                )
                pend = []
                for jc in range(JC):
                    eT = qk_exp(h, jc)
                    for fn in filler.get(jc, ()):
                        fn()
                    pend.append((jc, eT))
                    if jc >= 2:
                        j2, e2 = pend.pop(0)
                        pv(h, j2, e2)
                for j2, e2 in pend:
                    pv(h, j2, e2)

            # h0: remaining v chunks; kT(0,1) in jh-major granules paced by
            # the split memt half-1 arrival (the copy must precede QK(0,8))
            attn_head(0, {
                0: [lambda: v_chunk(6)],
                1: [lambda: v_chunk(7)],
                2: [lambda: v_chunk(8)],
                3: [lambda: v_chunk(9)],
                4: [lambda: v_chunk(10), lambda: kt_sub_jh(0, 1, 0)],
                5: [lambda: v_chunk(11)],
                6: [lambda: kt_sub_jh(0, 1, 1)],
                7: [lambda: kt_copy(0, 1)],
                8: [lambda: v_chunk(12)],
                9: [lambda: v_chunk(13)],
                10: [lambda: v_chunk(14)],
                11: [lambda: v_chunk(15)],
            })

            # h1 (odd head: QK uses PE rows 64-127, so the row 0-63 proj
            # matmuls overlap on the array): evac h0, kT(1,0), proj h0
            attn_head(1, {
                0: [lambda: ctx_evac(0), lambda: kt_sub(1, 0, 0)],
                1: [lambda: den_dma(0), lambda: kt_sub(1, 0, 1)],
                2: [lambda: kt_sub(1, 0, 2)],
                3: [lambda: kt_sub(1, 0, 3), lambda: kt_copy(1, 0)],
                4: [lambda: proj(0, 0)],
                5: [lambda: proj(0, 1)],
                6: [lambda: proj(0, 2)],
                7: [lambda: proj(0, 3)],
                8: [lambda: proj(0, 4)],
                9: [lambda: proj(0, 5)],
                10: [lambda: proj(0, 6)],
                11: [lambda: proj(0, 7)],
            })

            # h2 (even): evac h1, kT(1,1); otherwise ACT-paced
            attn_head(2, {
                0: [lambda: ctx_evac(1), lambda: kt_sub(1, 1, 0)],
                1: [lambda: den_dma(1), lambda: kt_sub(1, 1, 1)],
                2: [lambda: kt_sub(1, 1, 2)],
                3: [lambda: kt_sub(1, 1, 3), lambda: kt_copy(1, 1)],
            })

            # h3 (odd): evac h2, h1 + h2 projections (again row-overlapped)
            attn_head(3, {
                0: [lambda: ctx_evac(2), lambda: proj(1, 0)],
                1: [lambda: den_dma(2), lambda: proj(1, 1)],
                2: [lambda: proj(1, 2)],
                3: [lambda: proj(1, 3)],
                4: [lambda: proj(1, 4)],
                5: [lambda: proj(1, 5)],
                6: [lambda: proj(1, 6)],
                7: [lambda: proj(1, 7)],
                8: [lambda: proj(2, 0)],
                9: [lambda: proj(2, 1)],
                10: [lambda: proj(2, 2)],
                11: [lambda: proj(2, 3)],
                12: [lambda: proj(2, 4)],
                13: [lambda: proj(2, 5)],
                14: [lambda: proj(2, 6)],
                15: [lambda: proj(2, 7)],
            })

            # tail: evac h3 (bridged by dummies so the 1.2us CAST wait never
            # trips the HAM idle window), project, alternate evacuation
            # between DVE and the now-idle ACT
            ctx_evac(3)
            den_dma(3)
            dummy_mm(4)
            for ic in range(8):
                proj(3, ic, evac=("vector" if ic % 2 == 0 else "scalar"))
                if ic in (2, 5):
                    dummy_mm(1)

    nc.compile()
    return nc


def get_module():
    if "nc" not in _CACHE:
        _CACHE["nc"] = _build_module()
    return _CACHE["nc"]


def make_in_maps(x, mem, W_kv, W_q, W_proj):
    """Host-side shard + repack into the k-major bf16 layouts."""
    bf = ml_dtypes.bfloat16
    x = np.ascontiguousarray(np.asarray(x, np.float32))
    mem = np.ascontiguousarray(np.asarray(mem, np.float32))
    W_kv = np.asarray(W_kv, np.float32)
    W_q = np.asarray(W_q, np.float32)
    W_proj = np.asarray(W_proj, np.float32)

    def pack_k(a):  # [1024, N] -> [128, 8, N] bf16, k-chunked
        n = a.shape[1]
        return (
            np.ascontiguousarray(a.reshape(KC, 128, n).transpose(1, 0, 2))
            .astype(bf)
        )

    xt_b = [pack_k(x[b].T) for b in range(B)]
    # memt: [128, jh2, kc, NKV//2], kv-half major
    memt_b = [
        np.ascontiguousarray(
            pack_k(mem[b].T).reshape(128, KC, 2, NKV // 2).transpose(0, 2, 1, 3)
        )
        for b in range(B)
    ]

    in_maps = []
    for core in range(N_CORES):
        b, g = divmod(core, 4)
        cs = slice(g * DD, (g + 1) * DD)
        wq = pack_k(W_q[:, cs])
        wk = pack_k(W_kv[:, :DIM][:, cs])
        wv = pack_k(W_kv[:, DIM:][:, cs])
        # wp[d, h, n] = W_proj[g*256 + h*64 + d, n]
        wp = (
            np.ascontiguousarray(
                W_proj[cs, :].reshape(HG, HD, DIM).transpose(1, 0, 2)
            )
            .astype(bf)
        )
        in_maps.append(
            {
                "xt": xt_b[b],
                "memt": memt_b[b],
                "wq": wq,
                "wk": wk,
                "wv": wv,
                "wp": wp,
            }
        )
    return in_maps


def combine_outputs(results, b_proj):
    """Host: per-head softmax division + sum partials + bias."""
    b_proj = np.asarray(b_proj, np.float32)
    out = np.zeros((B, NQ, DIM), np.float32)
    for core in range(N_CORES):
        pout = np.asarray(results[core]["pout"], np.float32)  # [HG, NQ, DIM]
        dens = np.asarray(results[core]["dens"], np.float32)  # [HG, NQ]
        out[core // 4] += np.einsum("hqd,hq->qd", pout, 1.0 / dens)
    out += b_proj[None, None, :]
    return out


def kernel(x, mem, W_kv, W_q, W_proj, b_proj):
    from concourse import bass_utils

    nc = get_module()
    in_maps = make_in_maps(x, mem, W_kv, W_q, W_proj)
    res = bass_utils.run_bass_kernel_spmd(
        nc, in_maps, core_ids=list(range(N_CORES))
    )
    return combine_outputs([res.results[c] for c in range(N_CORES)], b_proj)


# revision 30
# speedup vs baseline: 1.0423x; 1.0007x over previous
"""Trainium2 Bass kernel: CrossAttention (B=2, Nq=1024, Nkv=2048, D=1024, H=16).

Sharding: 8 cores = 2 (batch) x 4 (head groups of 4 heads).
Each core computes, for its batch b and heads [4g, 4g+4):
    qT = (x_b @ Wq_s)^T            [256, 1024]   (dd on partitions)
    kT = (mem_b @ Wk_s)^T          [256, 2048]
    v  = mem_b @ Wv_s              [2048, 256]   (+ ones column per head)
    per head: sT = k_h @ q_h^T     [2048, 1024]  (kv on partitions)
              eT = exp(SCALE*sT)
              cu = [v_h | 1]^T-acc [65, 1024]    (row 64 = softmax denom)
    per head: P_h = cu[0:64]^T-proj @ Wp_h       [1024, 1024] UNNORMALIZED
Host does the softmax division: out = sum_h P_h / den_h + b_proj.
(Deferring the normalization to the host removes the recip/broadcast/mul
chains that idled the PE at head boundaries and triggered HAM re-throttle.)

v4 scheduling notes:
 - HAM discipline: PE gaps >~1.4us re-throttle the clock to 1.2 GHz and it
   costs ~3.4us of dense work to re-warm.  Dummy matmuls on a memset tile
   warm the PE from t=0 and bridge DMA stalls in the startup phase.
 - DMA order is by first-need: wq/xt stream qT, wk+memt half-0 feed
   kT(0,0), the rest follows.  ones/vones DMAs replaced with memsets.
 - Outputs are bf16 per-head unnormalized projections, DMA'd per-row-chunk
   as soon as each evacuation finishes (no f32 combine on device, no
   serial DVE add tail).
 - ACT only runs Exp (+Copy at the tail): table loaded at t=0.
"""

import numpy as np
import ml_dtypes

DIM = 1024
HEADS = 16
HD = 64
B = 2
NQ = 1024
NKV = 2048
SCALE = HD ** -0.5
N_CORES = 8
HG = 4               # heads per core
DD = HG * HD         # 256 packed head dims per core
KC = 8               # contraction chunks (DIM / 128)
JC = NKV // 128      # 16 kv-row chunks

_CACHE = {}


def _build_module():
    import concourse.bacc as bacc
    import concourse.tile as tile
    import concourse.mybir as mybir

    f32 = mybir.dt.float32
    bf16 = mybir.dt.bfloat16
    EXP = mybir.ActivationFunctionType.Exp

    nc = bacc.Bacc(
        trn_type="TRN2",
        target_bir_lowering=False,
        debug=False,
        num_devices=N_CORES,
    )

    xt_d = nc.dram_tensor("xt", [128, KC, NQ], bf16, kind="ExternalInput").ap()
    memt_d = nc.dram_tensor(
        "memt", [128, 2, KC, NKV // 2], bf16, kind="ExternalInput"
    ).ap()
    wq_d = nc.dram_tensor("wq", [128, KC, DD], bf16, kind="ExternalInput").ap()
    wk_d = nc.dram_tensor("wk", [128, KC, DD], bf16, kind="ExternalInput").ap()
    wv_d = nc.dram_tensor("wv", [128, KC, DD], bf16, kind="ExternalInput").ap()
    wp_d = nc.dram_tensor("wp", [64, HG, DIM], bf16, kind="ExternalInput").ap()
    pout_d = nc.dram_tensor("pout", [HG, NQ, DIM], bf16, kind="ExternalOutput").ap()
    dens_d = nc.dram_tensor("dens", [HG, NQ], bf16, kind="ExternalOutput").ap()

    with tile.TileContext(nc) as tc:
        with (
            tc.tile_pool(name="wpool", bufs=1) as wpool,
            tc.tile_pool(name="persist", bufs=1) as persist,
            tc.tile_pool(name="work", bufs=2) as work,
            tc.tile_pool(name="opool", bufs=8) as opool,
            tc.tile_pool(name="psum", bufs=3, space="PSUM") as psum,
        ):
            # ---- t=0: engine warm-ups, no DMA dependencies ----
            dummy_sb = wpool.tile([128, 512], bf16, name="dummy_sb")
            nc.vector.memset(dummy_sb, 0.125)
            warm_sb = wpool.tile([1, 16], f32, name="warm_sb")
            nc.scalar.activation(out=warm_sb, in_=dummy_sb[0:1, 0:16], func=EXP)
            vaug_sb = persist.tile([128, JC, HG, HD + 1], bf16, name="vaug_sb")
            nc.vector.memset(vaug_sb, 1.0)

            ndum = [0]

            def dummy_mm(n=1):
                # PE heater: keeps the HAM activity window busy while the
                # PE would otherwise idle on DMA (never on the dep chain).
                for _ in range(n):
                    d_ps = psum.tile(
                        [128, 512], f32, name=f"d_ps{ndum[0]}", tag="ps", bufs=3
                    )
                    ndum[0] += 1
                    nc.tensor.matmul(
                        d_ps, lhsT=dummy_sb[:, 0:128], rhs=dummy_sb,
                        start=True, stop=True,
                    )

            dummy_mm(19)

            # ---- DMA issues, ordered by first need (sync = HWDGE FIFO).
            # Critical path to the first exp: qT needs wq+xt (2.5MB), kT(0,0)
            # needs wk+memt half-0 (2.5MB); emission below matches arrival ----
            wq_lo = wpool.tile([128, 4, DD], bf16, name="wq_lo")
            nc.sync.dma_start(out=wq_lo, in_=wq_d[:, 0:4, :])
            xt_t = []
            for i in range(4):
                t = wpool.tile([128, 2, NQ], bf16, name=f"xt{i}")
                xt_t.append(t)
            nc.sync.dma_start(out=xt_t[0], in_=xt_d[:, 0:2, :])
            wk_sb = wpool.tile([128, KC, DD], bf16, name="wk_sb")
            nc.sync.dma_start(out=wk_sb, in_=wk_d)
            memt0a = wpool.tile([128, 4, NKV // 2], bf16, name="memt0a")
            nc.sync.dma_start(out=memt0a, in_=memt_d[:, 0, 0:4, :])
            memt0b = wpool.tile([128, 4, NKV // 2], bf16, name="memt0b")
            nc.sync.dma_start(out=memt0b, in_=memt_d[:, 0, 4:8, :])
            wv_sb = wpool.tile([128, KC, DD], bf16, name="wv_sb")
            nc.sync.dma_start(out=wv_sb, in_=wv_d)
            nc.sync.dma_start(out=xt_t[1], in_=xt_d[:, 2:4, :])
            wq_hi = wpool.tile([128, 4, DD], bf16, name="wq_hi")
            nc.sync.dma_start(out=wq_hi, in_=wq_d[:, 4:8, :])
            nc.sync.dma_start(out=xt_t[2], in_=xt_d[:, 4:6, :])
            nc.sync.dma_start(out=xt_t[3], in_=xt_d[:, 6:8, :])
            memt1a = wpool.tile([128, KC, 512], bf16, name="memt1a")
            nc.sync.dma_start(out=memt1a, in_=memt_d[:, 1, :, 0:512])
            memt1b = wpool.tile([128, KC, 512], bf16, name="memt1b")
            nc.sync.dma_start(out=memt1b, in_=memt_d[:, 1, :, 512:1024])
            wp_sb = wpool.tile([64, HG, DIM], bf16, name="wp_sb")
            nc.sync.dma_start(out=wp_sb, in_=wp_d)

            def memt(jh2, kc, lo, hi):
                # column window [lo, hi) within the 1024-wide kv half
                if jh2 == 0:
                    t = memt0a if kc < 4 else memt0b
                    return t[:, kc % 4, lo:hi]
                if hi <= 512:
                    return memt1a[:, kc, lo:hi]
                return memt1b[:, kc, lo - 512 : hi - 512]

            def wq(kc):
                return wq_lo[:, kc, :] if kc < 4 else wq_hi[:, kc - 4, :]

            # ---- persistent intermediates ----
            qT_sb = persist.tile([128, 2, NQ], bf16, name="qT_sb")
            kT_sb = persist.tile([128, 2, NKV], bf16, name="kT_sb")
            ctxr = [None] * HG
            cu = [None] * HG

            kt_state = {}

            def kt_tile(mc, jh2, tag="ps"):
                key = (mc, jh2)
                kt_ps = kt_state.get(key)
                if kt_ps is None:
                    kt_ps = psum.tile(
                        [128, NKV // 2], f32, name=f"kt_ps_{mc}_{jh2}",
                        tag=tag, bufs=(1 if tag == "cu" else 3),
                    )
                    kt_state[key] = kt_ps
                return kt_ps

            def kt_sub(mc, jh2, ki, tag="ps"):
                # kT quarter in kc-pair granules (4 mms each)
                kt_ps = kt_tile(mc, jh2, tag)
                for kc in (2 * ki, 2 * ki + 1):
                    for jh in range(2):
                        nc.tensor.matmul(
                            kt_ps[:, jh * 512 : (jh + 1) * 512],
                            lhsT=wk_sb[:, kc, mc * 128 : (mc + 1) * 128],
                            rhs=memt(jh2, kc, jh * 512, (jh + 1) * 512),
                            start=(kc == 0),
                            stop=(kc == KC - 1),
                        )

            def kt_sub_jh(mc, jh2, jh, tag="ps"):
                # kT quarter in jh-major granules (8 mms, one column bank):
                # lets kv-half-1 work start as soon as memt1a lands
                kt_ps = kt_tile(mc, jh2, tag)
                for kc in range(KC):
                    nc.tensor.matmul(
                        kt_ps[:, jh * 512 : (jh + 1) * 512],
                        lhsT=wk_sb[:, kc, mc * 128 : (mc + 1) * 128],
                        rhs=memt(jh2, kc, jh * 512, (jh + 1) * 512),
                        start=(kc == 0),
                        stop=(kc == KC - 1),
                    )

            def kt_copy(mc, jh2):
                kt_ps = kt_state.pop((mc, jh2))
                nc.vector.tensor_copy(
                    out=kT_sb[:, mc, jh2 * 1024 : (jh2 + 1) * 1024], in_=kt_ps
                )

            # ---- qT + kT(0,0), interleaved to match DMA arrival order:
            # qt kc0-1 (wq_lo+xt0), kT(0,0) (wk+memt half-0, parked in the
            # idle cu psum slot), then qt kc2-7 as the xt chunks land ----
            qt_ps = [
                psum.tile([128, NQ], f32, name=f"qt_ps{mc}", tag="ps", bufs=3)
                for mc in range(2)
            ]

            def qt_kc(kc, mcs=(0, 1)):
                xs = xt_t[kc // 2][:, kc % 2, :]
                for mc in mcs:
                    for ih in range(2):
                        nc.tensor.matmul(
                            qt_ps[mc][:, ih * 512 : (ih + 1) * 512],
                            lhsT=wq(kc)[:, mc * 128 : (mc + 1) * 128],
                            rhs=xs[:, ih * 512 : (ih + 1) * 512],
                            start=(kc == 0),
                            stop=(kc == KC - 1),
                        )

            def v_chunk(jc):
                jh2, jr = divmod(jc, 8)
                v_ps = psum.tile([128, DD], f32, name=f"v_ps{jc}", tag="ps", bufs=3)
                for kc in range(KC):
                    nc.tensor.matmul(
                        v_ps,
                        lhsT=memt(jh2, kc, jr * 128, (jr + 1) * 128),
                        rhs=wv_sb[:, kc, :],
                        start=(kc == 0),
                        stop=(kc == KC - 1),
                    )
                nc.vector.tensor_copy(
                    out=vaug_sb[:, jc, :, 0:HD],
                    in_=v_ps.rearrange("p (h d) -> p h d", h=HG),
                )

            def qk_exp(h, jc):
                hp = h // 2
                po = (h % 2) * 64
                sT = psum.tile(
                    [128, NQ], f32, name=f"sT_ps_{h}_{jc}", tag="ps", bufs=3
                )
                for ih in range(2):
                    nc.tensor.matmul(
                        sT[:, ih * 512 : (ih + 1) * 512],
                        lhsT=kT_sb[po : po + 64, hp, jc * 128 : (jc + 1) * 128],
                        rhs=qT_sb[po : po + 64, hp, ih * 512 : (ih + 1) * 512],
                        start=True,
                        stop=True,
                    )
                eT = work.tile([128, NQ], bf16, name="eT_sb", tag="eT", bufs=6)
                nc.scalar.activation(out=eT, in_=sT, func=EXP, scale=SCALE)
                return eT

            def pv(h, jc, eT):
                for ih in range(2):
                    nc.tensor.matmul(
                        cu[h][:, ih * 512 : (ih + 1) * 512],
                        lhsT=vaug_sb[:, jc, h, :],
                        rhs=eT[:, ih * 512 : (ih + 1) * 512],
                        start=(jc == 0),
                        stop=(jc == JC - 1),
                    )

            def ctx_evac(h):
                # single CAST frees the cu psum slot; row 64 is the denom
                ctxr[h] = persist.tile([65, NQ], bf16, name=f"ctxr{h}")
                nc.vector.tensor_copy(out=ctxr[h], in_=cu[h])

            def den_dma(h):
                nc.sync.dma_start(out=dens_d[h, :], in_=ctxr[h][64:65, :])

            def proj(h, ic, evac="vector"):
                p_ps = psum.tile(
                    [128, DIM], f32, name=f"p_ps_{h}_{ic}", tag="ps", bufs=3
                )
                for nh in range(2):
                    nc.tensor.matmul(
                        p_ps[:, nh * 512 : (nh + 1) * 512],
                        lhsT=ctxr[h][0:64, ic * 128 : (ic + 1) * 128],
                        rhs=wp_sb[:, h, nh * 512 : (nh + 1) * 512],
                        start=True,
                        stop=True,
                    )
                o_sb = opool.tile([128, DIM], bf16, name="o_sb", tag="out")
                if evac == "vector":
                    nc.vector.tensor_copy(out=o_sb, in_=p_ps)
                else:
                    nc.scalar.copy(out=o_sb, in_=p_ps)
                nc.sync.dma_start(
                    out=pout_d[h, ic * 128 : (ic + 1) * 128, :], in_=o_sb
                )

            # ---- startup emission: qt kc0-1, kT(0,0), then v chunks 0-5
            # fill the rest of the DMA-bound window (wv lands right after
            # memt half-0); qt resumes per xt arrival ----
            qt_kc(0)
            qt_kc(1)
            dummy_mm(1)
            for ki in range(4):
                kt_sub(0, 0, ki, tag="cu")
                if ki in (1, 2):
                    dummy_mm(1)
            kt_copy(0, 0)
            dummy_mm(2)
            v_chunk(0)
            v_chunk(1)
            qt_kc(2)
            qt_kc(3)
            v_chunk(2)
            v_chunk(3)
            qt_kc(4)
            qt_kc(5)
            qt_kc(6)
            # mc0 half of kc7 first: h0 only needs the mc=0 cast
            qt_kc(KC - 1, mcs=(0,))
            nc.vector.tensor_copy(out=qT_sb[:, 0, :], in_=qt_ps[0])
            qt_kc(KC - 1, mcs=(1,))
            nc.vector.tensor_copy(out=qT_sb[:, 1, :], in_=qt_ps[1])
            v_chunk(4)
            v_chunk(5)

            # ---- attention: heads sequential, PE filler interleaved ----
            def attn_head(h, filler):
                cu[h] = psum.tile(
                    [HD + 1, NQ], f32, name=f"cu_ps{h}", tag="cu", bufs=1
                )
                pend = []
                for jc in range(JC):
                    eT = qk_exp(h, jc)
                    for fn in filler.get(jc, ()):
                        fn()
                    pend.append((jc, eT))
                    if jc >= 2:
                        j2, e2 = pend.pop(0)
                        pv(h, j2, e2)
                for j2, e2 in pend:
                    pv(h, j2, e2)

            # h0: remaining v chunks; kT(0,1) in jh-major granules paced by
            # the split memt half-1 arrival (the copy must precede QK(0,8))
            attn_head(0, {
                0: [lambda: v_chunk(6)],
                1: [lambda: v_chunk(7)],
                2: [lambda: v_chunk(8)],
                3: [lambda: v_chunk(9)],
                4: [lambda: v_chunk(10), lambda: kt_sub_jh(0, 1, 0)],
                5: [lambda: v_chunk(11)],
                6: [lambda: kt_sub_jh(0, 1, 1)],
                7: [lambda: kt_copy(0, 1)],
                8: [lambda: v_chunk(12)],
                9: [lambda: v_chunk(13)],
                10: [lambda: v_chunk(14)],
                11: [lambda: v_chunk(15)],
            })

            # h1 (odd head: QK uses PE rows 64-127, so the row 0-63 proj
            # matmuls overlap on the array): evac h0, kT(1,0), proj h0
            attn_head(1, {
                0: [lambda: ctx_evac(0), lambda: kt_sub(1, 0, 0)],
                1: [lambda: den_dma(0), lambda: kt_sub(1, 0, 1)],
                2: [lambda: kt_sub(1, 0, 2)],
                3: [lambda: kt_sub(1, 0, 3), lambda: kt_copy(1, 0)],
                4: [lambda: proj(0, 0)],
                5: [lambda: proj(0, 1)],
                6: [lambda: proj(0, 2)],
                7: [lambda: proj(0, 3)],
                8: [lambda: proj(0, 4)],
                9: [lambda: proj(0, 5)],
                10: [lambda: proj(0, 6)],
                11: [lambda: proj(0, 7)],
            })

            # h2 (even): evac h1, kT(1,1); otherwise ACT-paced
            attn_head(2, {
                0: [lambda: ctx_evac(1), lambda: kt_sub(1, 1, 0)],
                1: [lambda: den_dma(1), lambda: kt_sub(1, 1, 1)],
                2: [lambda: kt_sub(1, 1, 2)],
                3: [lambda: kt_sub(1, 1, 3), lambda: kt_copy(1, 1)],
            })

            # h3 (odd): evac h2, h1 + h2 projections (again row-overlapped)
            attn_head(3, {
                0: [lambda: ctx_evac(2), lambda: proj(1, 0)],
                1: [lambda: den_dma(2), lambda: proj(1, 1)],
                2: [lambda: proj(1, 2)],
                3: [lambda: proj(1, 3)],
                4: [lambda: proj(1, 4)],
                5: [lambda: proj(1, 5)],
                6: [lambda: proj(1, 6)],
                7: [lambda: proj(1, 7)],
                8: [lambda: proj(2, 0)],
                9: [lambda: proj(2, 1)],
                10: [lambda: proj(2, 2)],
                11: [lambda: proj(2, 3)],
                12: [lambda: proj(2, 4)],
                13: [lambda: proj(2, 5)],
                14: [lambda: proj(2, 6)],
                15: [lambda: proj(2, 7)],
            })

            # tail: evac h3 (bridged by dummies so the 1.2us CAST wait never
            # trips the HAM idle window), project, alternate evacuation
            # between DVE and the now-idle ACT
            ctx_evac(3)
            den_dma(3)
            dummy_mm(4)
            for ic in range(8):
                proj(3, ic, evac=("vector" if ic % 2 == 0 else "scalar"))
                if ic in (2, 5):
                    dummy_mm(1)

    nc.compile()
    return nc


def get_module():
    if "nc" not in _CACHE:
        _CACHE["nc"] = _build_module()
    return _CACHE["nc"]


def make_in_maps(x, mem, W_kv, W_q, W_proj):
    """Host-side shard + repack into the k-major bf16 layouts."""
    bf = ml_dtypes.bfloat16
    x = np.ascontiguousarray(np.asarray(x, np.float32))
    mem = np.ascontiguousarray(np.asarray(mem, np.float32))
    W_kv = np.asarray(W_kv, np.float32)
    W_q = np.asarray(W_q, np.float32)
    W_proj = np.asarray(W_proj, np.float32)

    def pack_k(a):  # [1024, N] -> [128, 8, N] bf16, k-chunked
        n = a.shape[1]
        return (
            np.ascontiguousarray(a.reshape(KC, 128, n).transpose(1, 0, 2))
            .astype(bf)
        )

    xt_b = [pack_k(x[b].T) for b in range(B)]
    # memt: [128, jh2, kc, NKV//2], kv-half major
    memt_b = [
        np.ascontiguousarray(
            pack_k(mem[b].T).reshape(128, KC, 2, NKV // 2).transpose(0, 2, 1, 3)
        )
        for b in range(B)
    ]

    in_maps = []
    for core in range(N_CORES):
        b, g = divmod(core, 4)
        cs = slice(g * DD, (g + 1) * DD)
        wq = pack_k(W_q[:, cs])
        wk = pack_k(W_kv[:, :DIM][:, cs])
        wv = pack_k(W_kv[:, DIM:][:, cs])
        # wp[d, h, n] = W_proj[g*256 + h*64 + d, n]
        wp = (
            np.ascontiguousarray(
                W_proj[cs, :].reshape(HG, HD, DIM).transpose(1, 0, 2)
            )
            .astype(bf)
        )
        in_maps.append(
            {
                "xt": xt_b[b],
                "memt": memt_b[b],
                "wq": wq,
                "wk": wk,
                "wv": wv,
                "wp": wp,
            }
        )
    return in_maps


def combine_outputs(results, b_proj):
    """Host: per-head softmax division + sum partials + bias."""
    b_proj = np.asarray(b_proj, np.float32)
    out = np.zeros((B, NQ, DIM), np.float32)
    for core in range(N_CORES):
        pout = np.asarray(results[core]["pout"], np.float32)  # [HG, NQ, DIM]
        dens = np.asarray(results[core]["dens"], np.float32)  # [HG, NQ]
        out[core // 4] += np.einsum("hqd,hq->qd", pout, 1.0 / dens)
    out += b_proj[None, None, :]
    return out


def kernel(x, mem, W_kv, W_q, W_proj, b_proj):
    from concourse import bass_utils

    nc = get_module()
    in_maps = make_in_maps(x, mem, W_kv, W_q, W_proj)
    res = bass_utils.run_bass_kernel_spmd(
        nc, in_maps, core_ids=list(range(N_CORES))
    )
    return combine_outputs([res.results[c] for c in range(N_CORES)], b_proj)
